# revision 16
# baseline (speedup 1.0000x reference)
"""Trainium2 Bass kernel for Llama GQA attention (no mask), 8-way tensor
parallel over KV heads.

Problem shapes (hardcoded):
  x  (2, 2048, 4096) f32
  wq (4096, 4096), wk (1024, 4096), wv (1024, 4096), wo (4096, 4096) f32
  NUM_HEADS=32, NUM_KV_HEADS=8, HEAD_DIM=128, GQA group g=4

Sharding: core c owns KV head c (4 Q heads). x replicated (pre-transposed
to xT on host), wq/wk/wv sharded on output dim (pre-transposed host-side),
wo sharded on input dim. Each core computes a partial (4096, 4096) output
(its heads' contribution through wo); host sums the 8 partials.

v2 design notes (the kernel is pure PE-throughput bound, so the wins are
fewer streamed columns):
  - The softmax denominator no longer gets its own 512-wide matmul pass.
    PV is computed with p as the STATIONARY operand (bf16, full rate at
    any width) against a ones-augmented V [128, 129]; column 128 of the
    PSUM output is the denominator. This halves the PV+den column count.
  - The fused output is attn in natural [tq, dv] layout; cheap bf16 PE
    transposes (128 cols each) restore the [dv, tq] layout the output
    projection needs.
  - Out-projection runs in bf16 (attnT x wo), same 1 cyc/row as fp32r
    but with FWL weight loads.
  - Phase 2 is a flat software pipeline over (chunk, head, k-tile):
    the fp32r scores matmul runs one step ahead of the fused bf16
    PV+den matmuls, and the previous chunk's out-projection groups are
    interleaved every other step so the PE always has work while the
    scalar engine exps.
  - Two [128,129] accumulators share one PSUM bank (matmul start=True
    clears has_written for the whole bank; the second accumulator's
    first matmul uses start=False and relies on per-element
    overwrite-where-unset).
"""

import sys
from contextlib import ExitStack

import numpy as np

sys.path.insert(0, "/opt/trn_rl_repo")

import concourse.bass as bass  # noqa: E402
import concourse.tile as tile  # noqa: E402
from concourse import bacc, mybir  # noqa: E402
from concourse.bass_utils import run_bass_kernel_spmd  # noqa: E402
from concourse.masks import make_identity  # noqa: E402

NCORES = 8
B, S, H = 2, 2048, 4096
T = B * S                      # 4096 flattened tokens
D = 128                        # head dim
G = 4                          # q heads per core (GQA group)
HK = 32                        # h k-tiles (4096 / 128)
TT = T // 128                  # 32 token tiles
NJ = T // 512                  # 8 token chunks of 512
SJ = S // 512                  # 4 tq chunks per batch
SI = S // 128                  # 16 tk tiles per batch
CH = B * SJ                    # 8 (batch, tq-chunk) pairs
SCALE = float(1.0 / np.sqrt(D))

F32 = mybir.dt.float32
F32R = mybir.dt.float32r
BF16 = mybir.dt.bfloat16
COPY = mybir.ActivationFunctionType.Copy
EXP = mybir.ActivationFunctionType.Exp


def build_nc():
    nc = bacc.Bacc("TRN2", target_bir_lowering=False, debug=False,
                   enable_asserts=True, num_devices=NCORES)
    xt = nc.declare_dram_parameter("xt", [H, T], BF16, isOutput=False)
    wqt = nc.declare_dram_parameter("wqt", [H, G * D], BF16, isOutput=False)
    wkt = nc.declare_dram_parameter("wkt", [H, D], BF16, isOutput=False)
    wvt = nc.declare_dram_parameter("wvt", [H, D], BF16, isOutput=False)
    wot = nc.declare_dram_parameter("wot", [G * D, H], BF16, isOutput=False)
    out = nc.declare_dram_parameter("out", [T, H], F32, isOutput=True)

    xt_r = xt.ap().rearrange("(k p) t -> p k t", p=128)     # [128, 32, T]
    wqt_r = wqt.ap().rearrange("(k p) m -> p k m", p=128)   # [128, 32, 512]
    wkt_r = wkt.ap().rearrange("(k p) m -> p k m", p=128)   # [128, 32, 128]
    wvt_r = wvt.ap().rearrange("(k p) m -> p k m", p=128)   # [128, 32, 128]
    wot_r = wot.ap().rearrange("(k p) n -> p k n", p=128)   # [128, 4, T]
    out_r = out.ap()

    with tile.TileContext(nc) as tc:
        with ExitStack() as ctx:
            persist = ctx.enter_context(tc.tile_pool(name="persist", bufs=1))
            q_sb = persist.tile([128, G, T], BF16)       # qT per head, 4MB
            k_sb = persist.tile([128, T], BF16)          # kT, 1MB
            # v natural + ones column at 128 (cols 129-131 unused pad)
            v_sb = persist.tile([128, TT, 132], BF16)
            ident16 = persist.tile([128, 128], BF16)
            make_identity(nc, ident16)
            nc.vector.memset(v_sb[:, :, 128:129], 1.0)

            # warm the PE HAM clock gate during the initial DMA wait so the
            # first real matmuls run at 2.4 GHz instead of 1.2
            with tc.tile_pool(name="warm", bufs=1, space="PSUM") as wrm:
                w_ps = wrm.tile([128, 128], BF16)
                for _ in range(32):
                    nc.tensor.transpose(w_ps, ident16, ident16)

            # ---------------- phase 1: projections ----------------
            with ExitStack() as c1:
                wpool = c1.enter_context(tc.tile_pool(name="wpool", bufs=1))
                xpool = c1.enter_context(tc.tile_pool(name="xpool", bufs=6))
                x16pool = c1.enter_context(tc.tile_pool(name="x16", bufs=12))
                wstg = c1.enter_context(tc.tile_pool(name="wstg", bufs=4))
                vstg = c1.enter_context(tc.tile_pool(name="vstg", bufs=2))
                ps1 = c1.enter_context(tc.tile_pool(name="ps1", bufs=1, space="PSUM"))
                pstr = c1.enter_context(tc.tile_pool(name="pstr", bufs=2, space="PSUM"))

                wq_t = wpool.tile([128, HK, G * D], BF16)   # 4MB
                wk_t = wpool.tile([128, HK, D], BF16)       # 1MB
                wv_t = wpool.tile([128, HK, D], BF16)       # 1MB
                # weight k-chunks are paced: the first few load upfront,
                # the rest are issued inside j=0's k-loop with a 4-iteration
                # lead so they don't fight the x stream for HBM early on
                def load_w_chunk(k):
                    nc.gpsimd.dma_start(out=wq_t[:, k, :], in_=wqt_r[:, k, :])
                    nc.gpsimd.dma_start(out=wk_t[:, k, :], in_=wkt_r[:, k, :])
                    nc.gpsimd.dma_start(out=wv_t[:, k, :], in_=wvt_r[:, k, :])
                for k in range(4):
                    load_w_chunk(k)

                def v_transpose(pj, pv_st):
                    # one-j-delayed so PE never waits on the DVE staging copy
                    vt_ps = pstr.tile([128, 4, 128], BF16)
                    for tt in range(4):
                        nc.tensor.transpose(
                            vt_ps[:, tt, :], pv_st[:, tt * 128:(tt + 1) * 128],
                            ident16)
                    nc.scalar.activation(
                        out=v_sb[:, 4 * pj:4 * pj + 4, 0:128], in_=vt_ps,
                        func=COPY)

                prev_v = None
                for j in range(NJ):
                    tsl = slice(j * 512, (j + 1) * 512)
                    q_ps = [ps1.tile([128, 512], F32, name=f"q_ps{m}")
                            for m in range(G)]
                    k_ps = ps1.tile([128, 512], F32)
                    v_ps = ps1.tile([128, 512], F32)
                    for k in range(HK):
                        x16_t = x16pool.tile([128, 512], BF16)
                        nc.sync.dma_start(out=x16_t, in_=xt_r[:, k, tsl])
                        x_t = xpool.tile([128, 512], BF16)
                        nc.vector.tensor_copy(x_t, x16_t)
                        st = k == 0
                        sp = k == HK - 1
                        for m in range(G):
                            nc.tensor.matmul(
                                q_ps[m], wq_t[:, k, m * D:(m + 1) * D], x_t,
                                start=st, stop=sp)
                        nc.tensor.matmul(k_ps, wk_t[:, k, :], x_t, start=st, stop=sp)
                        nc.tensor.matmul(v_ps, wv_t[:, k, :], x_t, start=st, stop=sp)
                        if j == 0 and k + 6 < HK and k >= 2:
                            load_w_chunk(k + 6)
                        if j == 0 and k == 1:
                            load_w_chunk(4)
                            load_w_chunk(5)
                            load_w_chunk(6)
                            load_w_chunk(7)
                        if k == 2 and prev_v is not None:
                            v_transpose(*prev_v)
                    # split psum evacuation across ACT and DVE so the banks
                    # free up fast for the next j iteration
                    nc.scalar.activation(out=q_sb[:, 0, tsl], in_=q_ps[0], func=COPY)
                    nc.vector.tensor_copy(q_sb[:, 1, tsl], q_ps[1])
                    nc.scalar.activation(out=q_sb[:, 2, tsl], in_=q_ps[2], func=COPY)
                    nc.vector.tensor_copy(q_sb[:, 3, tsl], q_ps[3])
                    nc.scalar.activation(out=k_sb[:, tsl], in_=k_ps, func=COPY)
                    # v: vT [dv, t] -> transpose 128-col blocks -> v [t, dv]
                    v_st = vstg.tile([128, 512], BF16)
                    nc.vector.tensor_copy(v_st, v_ps)
                    prev_v = (j, v_st)
                v_transpose(*prev_v)

            # ------- phase 2: fused attention + output projection -------
            with ExitStack() as c2:
                wopool = c2.enter_context(tc.tile_pool(name="wopool", bufs=1))
                apool = c2.enter_context(tc.tile_pool(name="apool", bufs=2))
                ppool = c2.enter_context(tc.tile_pool(name="ppool", bufs=3))
                npool = c2.enter_context(tc.tile_pool(name="npool", bufs=2))
                rpool = c2.enter_context(tc.tile_pool(name="rpool", bufs=2))
                opool = c2.enter_context(tc.tile_pool(name="opool", bufs=3))
                psS = c2.enter_context(tc.tile_pool(name="psS", bufs=3, space="PSUM"))
                psF = c2.enter_context(tc.tile_pool(name="psF", bufs=1, space="PSUM"))
                psT = c2.enter_context(tc.tile_pool(name="psT", bufs=1, space="PSUM"))
                psO = c2.enter_context(tc.tile_pool(name="psO", bufs=2, space="PSUM"))

                wo_sb = wopool.tile([128, G, T], BF16)      # 4MB resident
                for k in range(G):
                    nc.gpsimd.dma_start(out=wo_sb[:, k, :], in_=wot_r[:, k, :])

                # per-chunk live state
                a_chT = {}     # chunk -> [4 tiles [128,512] bf16]
                o_acc = {}     # (chunk, m) -> (o_psA, o_psB)
                p_tiles = {}   # (chunk, m, ti) -> p_t
                s_tiles = {}   # (chunk, m, ti) -> s_ps

                iters = [(c, m, ti) for c in range(CH) for m in range(G)
                         for ti in range(SI)]
                N = len(iters)

                def tq_slice(c):
                    b, j = divmod(c, SJ)
                    return b, slice(b * S + j * 512, b * S + (j + 1) * 512)

                def emit_score(c, m, ti):
                    b, tqsl = tq_slice(c)
                    kti = b * SI + ti
                    s_ps = psS.tile([128, 512], F32)
                    nc.tensor.matmul(
                        s_ps, k_sb[:, kti * 128:(kti + 1) * 128],
                        q_sb[:, m, tqsl], start=True, stop=True)
                    p_t = ppool.tile([128, 512], BF16)
                    nc.scalar.activation(out=p_t, in_=s_ps, func=EXP,
                                         scale=SCALE)
                    p_tiles[(c, m, ti)] = p_t

                def emit_fused(c, m, ti):
                    b, _ = tq_slice(c)
                    kti = b * SI + ti
                    if ti == 0:
                        o_psA = psF.tile([128, 2, 132], F32, name="o_psA")
                        o_psB = psF.tile([128, 2, 132], F32, name="o_psB")
                        o_acc[(c, m)] = (o_psA, o_psB)
                    o_psA, o_psB = o_acc[(c, m)]
                    p_t = p_tiles.pop((c, m, ti))
                    for tt2 in range(4):
                        jj, col = divmod(tt2, 2)
                        ops = o_psA if jj == 0 else o_psB
                        first = ti == 0 and col == 0
                        nc.tensor.matmul(
                            ops[:, col, 0:129],
                            p_t[:, tt2 * 128:(tt2 + 1) * 128],
                            v_sb[:, kti, 0:129],
                            start=first, stop=(ti == SI - 1),
                            skip_group_check=(col == 1))
                    if ti == SI - 1:
                        # denominator is column 128 of each accumulator;
                        # scale attn rows by its reciprocal (DVE)
                        rec_t = rpool.tile([128, 4], F32)
                        nc.vector.reciprocal_approx_fast(
                            out=rec_t[:, 0:2], in_=o_psA[:, :, 128:129])
                        nc.vector.reciprocal_approx_fast(
                            out=rec_t[:, 2:4], in_=o_psB[:, :, 128:129])
                        attn_nat = npool.tile([128, 4, 128], BF16)
                        for tt2 in range(4):
                            jj, col = divmod(tt2, 2)
                            ops = o_psA if jj == 0 else o_psB
                            nc.vector.tensor_scalar_mul(
                                attn_nat[:, tt2, :], ops[:, col, 0:128],
                                rec_t[:, tt2:tt2 + 1])
                        return attn_nat
                    return None

                def emit_transpose(c, m, attn_nat):
                    tr_ps = psT.tile([128, 4, 128], BF16)
                    for tt2 in range(4):
                        nc.tensor.transpose(
                            tr_ps[:, tt2, :], attn_nat[:, tt2, :], ident16)
                    if c not in a_chT:
                        a_chT[c] = [apool.tile([128, 512], BF16, name=f"a{mm}")
                                    for mm in range(G)]
                    nc.scalar.activation(out=a_chT[c][m], in_=tr_ps, func=COPY)

                def emit_outproj(c, g, tail=False):
                    # one (tt2, n) group: 4 bf16 matmuls accumulating over m.
                    # In the tail, each stationary serves two n-columns
                    # (halved weight loads) and psS/psF banks are borrowed
                    # for deeper accumulator rotation; ACT shares evictions.
                    b, j = divmod(c, SJ)
                    tt2, n = divmod(g, NJ)
                    t0 = b * S + j * 512 + tt2 * 128
                    ns = (n,) if not tail else (n, n + 1)
                    pools = {0: lambda: psO.tile([128, 512], F32, name="o_ps"),
                             1: lambda: psS.tile([128, 512], F32, name="s_ps")}
                    o_pss = [pools[(n + i) % 2 if tail else 0]()
                             for i in range(len(ns))]
                    pa = a_chT[c]
                    for m in range(G):
                        for o_ps, nn in zip(o_pss, ns):
                            nc.tensor.matmul(
                                o_ps, pa[m][:, tt2 * 128:(tt2 + 1) * 128],
                                wo_sb[:, m, nn * 512:(nn + 1) * 512],
                                start=(m == 0), stop=(m == G - 1))
                    for o_ps, nn in zip(o_pss, ns):
                        o_t = opool.tile([128, 512], F32)
                        if tail and nn % 2 == 0:
                            nc.scalar.activation(out=o_t, in_=o_ps, func=COPY)
                        else:
                            nc.vector.tensor_copy(o_t, o_ps)
                        nc.sync.dma_start(
                            out=out_r[t0:t0 + 128, nn * 512:(nn + 1) * 512],
                            in_=o_t)

                # slot -> list of deferred actions
                deferred = {}

                def defer(slot, fn):
                    deferred.setdefault(slot, []).append(fn)

                for c in range(CH - 1):
                    base = (c + 1) * 64
                    for g in range(4 * NJ):
                        defer(base + 2 * g + 7, (emit_outproj, c, g))

                for gi in range(N + 1):
                    if gi < N:
                        emit_score(*iters[gi])
                    if gi >= 1:
                        c, m, ti = iters[gi - 1]
                        attn_nat = emit_fused(c, m, ti)
                        if attn_nat is not None:
                            defer(gi + 2, (emit_transpose, c, m, attn_nat))
                    for item in deferred.pop(gi, ()):
                        item[0](*item[1:])
                # drain remaining deferred work (last chunk's transposes +
                # out-projection)
                for slot in sorted(deferred):
                    for item in deferred[slot]:
                        item[0](*item[1:])
                for tt2 in range(4):
                    for n in range(0, NJ, 2):
                        emit_outproj(CH - 1, tt2 * NJ + n, tail=True)
    nc.compile()
    return nc


_NC_CACHE = None


def _get_nc():
    global _NC_CACHE
    if _NC_CACHE is None:
        _NC_CACHE = build_nc()
    return _NC_CACHE


def make_in_maps(x, wq, wk, wv, wo):
    import ml_dtypes
    xt = np.ascontiguousarray(x.reshape(T, H).T).astype(ml_dtypes.bfloat16)
    in_maps = []
    for c in range(NCORES):
        qsl = slice(c * G * D, (c + 1) * G * D)
        ksl = slice(c * D, (c + 1) * D)
        in_maps.append({
            "xt": xt,
            "wqt": np.ascontiguousarray(wq[qsl, :].T).astype(ml_dtypes.bfloat16),
            "wkt": np.ascontiguousarray(wk[ksl, :].T).astype(ml_dtypes.bfloat16),
            "wvt": np.ascontiguousarray(wv[ksl, :].T).astype(ml_dtypes.bfloat16),
            "wot": np.ascontiguousarray(wo[:, qsl].T).astype(ml_dtypes.bfloat16),
        })
    return in_maps


def kernel(x, wq, wk, wv, wo, **run_kwargs):
    nc = _get_nc()
    in_maps = make_in_maps(np.asarray(x, dtype=np.float32),
                           np.asarray(wq, dtype=np.float32),
                           np.asarray(wk, dtype=np.float32),
                           np.asarray(wv, dtype=np.float32),
                           np.asarray(wo, dtype=np.float32))
    res = run_bass_kernel_spmd(nc, in_maps, core_ids=list(range(NCORES)),
                               **run_kwargs)
    acc = np.zeros((T, H), dtype=np.float32)
    for c in range(NCORES):
        acc += res.results[c]["out"]
    out = acc.reshape(B, S, H)
    if run_kwargs:
        return out, res
    return out
